# revision 3
# baseline (speedup 1.0000x reference)
"""CTC loss (keras ctc_batch_cost semantics) on 8 Trainium2 NeuronCores.

Strategy
--------
Data parallel over the batch: 8 cores x 64 examples each.

The CTC forward recursion runs in the probability domain (not log space):
    alpha_t[s] = (alpha_{t-1}[s] + alpha_{t-1}[s-1] + allow[s]*alpha_{t-1}[s-2])
                 * p_t[ext[s]]
with a periodic rescale every RENORM steps that renormalizes the per-example
total to K = 2**100 (keeping the profile peak near the top of the fp32 range
so ~150 nats of state-profile spread stay representable).  The per-step sums
C_t come for free out of the final multiply's fused accumulator; the host
reassembles  loss = -(log(alpha_T[S-1]+alpha_T[S-2]) + sum_k log(C_k/K) - logK)
in float64.

Device layout: batch in partitions (64 rows), states along the free dim with
2 zero-pad columns so the s-1/s-2 shifts are plain AP offsets.  Each step is
4 fused scalar_tensor_tensor ops on the vector engine.  The gathered
probability tensor P[b, t, s] = y_pred[b, t, ext[b, s]] + eps is built on the
host (numpy) and streamed to the device in double-buffered time chunks.
"""

import numpy as np

import concourse.bacc as bacc
import concourse.bass as bass
import concourse.tile as tile
from concourse import mybir
from concourse.bass_utils import run_bass_kernel_spmd

# Problem constants (hardcoded; kernel.py must be self-contained).
B, T, C, L = 512, 512, 128, 64
S = 2 * L + 1          # 129
BLANK = C - 1          # 127
EPS = 1e-7
NCORES = 8
BPC = B // NCORES      # 64 examples per core
RENORM = 4             # rescale cadence (steps)
K = float(2.0 ** 100)  # renormalization target for the per-example total
LOG_K = 100.0 * float(np.log(2.0))

F32 = mybir.dt.float32
MULT = mybir.AluOpType.mult
ADD = mybir.AluOpType.add


def build_nc(T_=T, TB=64, bpc=BPC, renorm=RENORM):
    """Build the Bass program. Parameterized so tests can build small variants."""
    nc = bacc.Bacc(
        "TRN2", target_bir_lowering=False, debug=False, num_devices=NCORES
    )
    P = nc.dram_tensor("P", [bpc, T_, S], F32, kind="ExternalInput")
    M = nc.dram_tensor("M", [bpc, S - 2], F32, kind="ExternalInput")
    XF = nc.dram_tensor("XF", [bpc, 2], F32, kind="ExternalOutput")
    CS = nc.dram_tensor("CS", [bpc, T_], F32, kind="ExternalOutput")

    Pap, Map, XFap, CSap = P.ap(), M.ap(), XF.ap(), CS.ap()
    nchunks = T_ // TB

    with tile.TileContext(nc) as tc:
        with (
            tc.tile_pool(name="persist", bufs=1) as pers,
            tc.tile_pool(name="pchunks", bufs=2) as pp,
        ):
            X = pers.tile([bpc, S + 2], F32)    # cols 0,1 = zero pad; 2..130 = states
            W = pers.tile([bpc, S], F32)
            G = pers.tile([bpc, S - 2], F32)
            m = pers.tile([bpc, S - 2], F32)
            Cs = pers.tile([bpc, T_], F32)
            rc = pers.tile([bpc, 1], F32)

            nc.vector.memset(X, 0.0)
            nc.sync.dma_start(out=m, in_=Map)

            for k in range(nchunks):
                pch = pp.tile([bpc, TB, S], F32, tag="pch")
                nc.sync.dma_start(out=pch, in_=Pap[:, k * TB : (k + 1) * TB, :])

                for i in range(TB):
                    tau = k * TB + i
                    pt = pch[:, i, :]  # [bpc, S]
                    if tau == 0:
                        # init: alpha_0[s] = K * p_0[ext[s]] for s in {0,1}
                        nc.vector.tensor_scalar_mul(X[:, 2:4], pt[:, 0:2], K)
                        nc.vector.tensor_reduce(
                            Cs[:, 0:1], X[:, 2:4], axis=mybir.AxisListType.X, op=ADD
                        )
                        continue
                    if tau % renorm == 0:
                        nc.vector.reciprocal(rc, Cs[:, tau - 1 : tau])
                        nc.vector.tensor_scalar_mul(rc, rc, K)
                        sc = rc[:, :]
                    else:
                        sc = 1.0
                    # W[s] = alpha[s-1] + alpha[s]
                    nc.vector.scalar_tensor_tensor(
                        W, X[:, 1 : S + 1], 1.0, X[:, 2 : S + 2], op0=MULT, op1=ADD
                    )
                    # G[s-2] = alpha[s-2] * allow[s]
                    nc.vector.scalar_tensor_tensor(
                        G, X[:, 2:S], 1.0, m, op0=MULT, op1=MULT
                    )
                    # W[s] += G  (s >= 2)
                    nc.vector.scalar_tensor_tensor(
                        W[:, 2:S], G, 1.0, W[:, 2:S], op0=MULT, op1=ADD
                    )
                    # alpha'[s] = (W[s] * sc) * p_t[s];  C[tau] = sum_s alpha'
                    nc.vector.scalar_tensor_tensor(
                        X[:, 2 : S + 2],
                        W,
                        sc,
                        pt,
                        op0=MULT,
                        op1=MULT,
                        accum_out=Cs[:, tau : tau + 1],
                    )

            nc.sync.dma_start(out=XFap, in_=X[:, S : S + 2])
            nc.sync.dma_start(out=CSap, in_=Cs)

    nc.compile()
    return nc


def host_build_inputs(y_true, y_pred, T_=T):
    """Build P [B, T, S] f32 and M [B, S-2] f32 on the host."""
    y_true = np.asarray(y_true).astype(np.int64)
    y_pred = np.asarray(y_pred).astype(np.float32)
    Bn = y_true.shape[0]
    p = y_pred + np.float32(EPS)
    ext = np.full((Bn, S), BLANK, dtype=np.int64)
    ext[:, 1::2] = y_true
    allow = np.zeros((Bn, S), dtype=bool)
    allow[:, 2:] = (ext[:, 2:] != BLANK) & (ext[:, 2:] != ext[:, :-2])
    P_full = np.take_along_axis(
        p[:, :T_, :], np.broadcast_to(ext[:, None, :], (Bn, T_, S)), axis=2
    ).astype(np.float32)
    M_full = np.ascontiguousarray(allow[:, 2:].astype(np.float32))
    return P_full, M_full


def host_finalize(XF, CS, T_=T, renorm=RENORM):
    """loss [n,1] f32 from device outputs (float64 on host)."""
    fin = XF[:, 0].astype(np.float64) + XF[:, 1].astype(np.float64)
    corr = -np.log(np.float64(K))
    for k in range(1, T_ // renorm):
        corr = corr + (
            np.log(CS[:, k * renorm - 1].astype(np.float64)) - np.log(np.float64(K))
        )
    loss = -(np.log(fin) + corr)
    return loss[:, None].astype(np.float32)


# test.py can set this to capture a hardware profile; the graded path leaves
# it off.
TRACE = False
LAST_RESULT = None
LAST_EXEC_S = None
_NC_CACHE = None


def kernel(y_true, y_pred):
    global LAST_RESULT, LAST_EXEC_S, _NC_CACHE
    import time as _time

    P_full, M_full = host_build_inputs(y_true, y_pred)
    if _NC_CACHE is None:
        _NC_CACHE = build_nc()
    nc = _NC_CACHE
    in_maps = [
        {
            "P": np.ascontiguousarray(P_full[c * BPC : (c + 1) * BPC]),
            "M": np.ascontiguousarray(M_full[c * BPC : (c + 1) * BPC]),
        }
        for c in range(NCORES)
    ]
    t0 = _time.time()
    res = run_bass_kernel_spmd(
        nc, in_maps, core_ids=list(range(NCORES)), trace=TRACE
    )
    LAST_EXEC_S = _time.time() - t0
    LAST_RESULT = res
    out = np.empty((B, 1), dtype=np.float32)
    for c in range(NCORES):
        r = res.results[c]
        out[c * BPC : (c + 1) * BPC] = host_finalize(r["XF"], r["CS"])
    return out


# revision 4
# speedup vs baseline: 3325.0018x; 3325.0018x over previous
"""CTC loss (keras ctc_batch_cost semantics) on 8 Trainium2 NeuronCores.

Strategy
--------
Pure data parallel over the batch: 8 cores x 64 examples each; no collectives.

The CTC forward recursion runs in the probability domain (not log space):
    alpha_t[s] = (alpha_{t-1}[s] + alpha_{t-1}[s-1] + allow[s]*alpha_{t-1}[s-2])
                 * p_t[ext[s]]
with a rescale every RENORM steps that renormalizes the per-example total to
K = 2**100, keeping the state-profile peak near the top of the fp32 exponent
range so ~150 nats of spread below the peak stay representable (a plain
renorm-to-1 loses ~1% of the probability mass to underflow; log-space per-step
logaddexp is far too slow on this hardware).  The per-step sums C_t come for
free from the final multiply's fused accumulator; the host reassembles
    loss = -(log(alpha_T[S-1] + alpha_T[S-2]) - logK + sum_k log(C_k/K))
in float64.  Storage is bf16 (DVE computes in fp32 internally; bf16 keeps the
fp32 exponent range), measured end-to-end max rel err ~1.1e-4.

Device layout: batch in partitions (64 rows), states along the free dim with
2 zero-pad columns so the s-1/s-2 shifts are plain AP offsets.  Each step is
4 fused scalar_tensor_tensor ops on the vector engine.  The gathered
probability tensor P[b, t, s] = y_pred[b, t, ext[b, s]] + eps is built on the
host (per-partition gathers are not expressible on-device: indirect_copy
shares its index stream across each 16-partition group) and streamed to the
device in double-buffered time chunks; P rows are padded to an even length so
per-step slices stay 4-byte aligned for the DVE 2x bf16 mode.
"""

import ml_dtypes
import numpy as np

import concourse.bacc as bacc
import concourse.bass as bass
import concourse.tile as tile
from concourse import mybir
from concourse.bass_utils import run_bass_kernel_spmd

B, T, C, L = 512, 512, 128, 64
S = 2 * L + 1
SP = S + 1             # P row padded to even length so per-step offsets stay 4B-aligned
BLANK = C - 1
EPS = 1e-7
NCORES = 8
BPC = B // NCORES
RENORM = 4
K = float(2.0 ** 100)
LOG_K = 100.0 * float(np.log(2.0))

F32 = mybir.dt.float32
BF16 = mybir.dt.bfloat16
MULT = mybir.AluOpType.mult
ADD = mybir.AluOpType.add


def build_nc(T_=T, TB=64, bpc=BPC, renorm=RENORM):
    nc = bacc.Bacc(
        "TRN2", target_bir_lowering=False, debug=False, num_devices=NCORES
    )
    P = nc.dram_tensor("P", [bpc, T_, SP], BF16, kind="ExternalInput")
    M = nc.dram_tensor("M", [bpc, S - 2], BF16, kind="ExternalInput")
    XF = nc.dram_tensor("XF", [bpc, 2], F32, kind="ExternalOutput")
    CS = nc.dram_tensor("CS", [bpc, T_], F32, kind="ExternalOutput")

    Pap, Map, XFap, CSap = P.ap(), M.ap(), XF.ap(), CS.ap()
    nchunks = T_ // TB

    with tile.TileContext(nc) as tc:
        with (
            tc.tile_pool(name="persist", bufs=1) as pers,
            tc.tile_pool(name="pchunks", bufs=2) as pp,
        ):
            X = pers.tile([bpc, S + 2], BF16)
            W = pers.tile([bpc, S], BF16)
            G = pers.tile([bpc, S - 2], BF16)
            m = pers.tile([bpc, S - 2], BF16)
            Cs = pers.tile([bpc, T_], F32)
            rc = pers.tile([bpc, 1], F32)
            xf32 = pers.tile([bpc, 2], F32)

            nc.vector.memset(X, 0.0)
            nc.sync.dma_start(out=m, in_=Map)

            for k in range(nchunks):
                pch = pp.tile([bpc, TB, SP], BF16, tag="pch")
                nc.sync.dma_start(out=pch, in_=Pap[:, k * TB : (k + 1) * TB, :])

                for i in range(TB):
                    tau = k * TB + i
                    pt = pch[:, i, 0:S]
                    if tau == 0:
                        nc.vector.tensor_scalar_mul(X[:, 2:4], pt[:, 0:2], K)
                        nc.vector.tensor_reduce(
                            Cs[:, 0:1], X[:, 2:4], axis=mybir.AxisListType.X, op=ADD
                        )
                        continue
                    if tau % renorm == 0:
                        nc.vector.reciprocal(rc, Cs[:, tau - 1 : tau])
                        nc.vector.tensor_scalar_mul(rc, rc, K)
                        sc = rc[:, :]
                    else:
                        sc = 1.0
                    nc.vector.scalar_tensor_tensor(
                        W, X[:, 1 : S + 1], 1.0, X[:, 2 : S + 2], op0=MULT, op1=ADD
                    )
                    nc.vector.scalar_tensor_tensor(
                        G, X[:, 2:S], 1.0, m, op0=MULT, op1=MULT
                    )
                    nc.vector.scalar_tensor_tensor(
                        W[:, 2:S], G, 1.0, W[:, 2:S], op0=MULT, op1=ADD
                    )
                    nc.vector.scalar_tensor_tensor(
                        X[:, 2 : S + 2],
                        W,
                        sc,
                        pt,
                        op0=MULT,
                        op1=MULT,
                        accum_out=Cs[:, tau : tau + 1],
                    )

            # upconvert the two final states to f32 for output
            nc.vector.tensor_copy(xf32, X[:, S : S + 2])
            nc.sync.dma_start(out=XFap, in_=xf32)
            nc.sync.dma_start(out=CSap, in_=Cs)

    nc.compile()
    return nc


def host_build_inputs(y_true, y_pred, T_=T):
    y_true = np.asarray(y_true).astype(np.int64)
    y_pred = np.asarray(y_pred).astype(np.float32)
    Bn = y_true.shape[0]
    p = y_pred + np.float32(EPS)
    ext = np.full((Bn, S), BLANK, dtype=np.int64)
    ext[:, 1::2] = y_true
    allow = np.zeros((Bn, S), dtype=bool)
    allow[:, 2:] = (ext[:, 2:] != BLANK) & (ext[:, 2:] != ext[:, :-2])
    P_full = np.zeros((Bn, T_, SP), dtype=ml_dtypes.bfloat16)
    P_full[:, :, :S] = np.take_along_axis(
        p[:, :T_, :], np.broadcast_to(ext[:, None, :], (Bn, T_, S)), axis=2
    ).astype(ml_dtypes.bfloat16)
    M_full = np.ascontiguousarray(allow[:, 2:].astype(ml_dtypes.bfloat16))
    return P_full, M_full


def host_finalize(XF, CS, T_=T, renorm=RENORM):
    fin = XF[:, 0].astype(np.float64) + XF[:, 1].astype(np.float64)
    corr = -np.log(np.float64(K))
    for k in range(1, T_ // renorm):
        corr = corr + (
            np.log(CS[:, k * renorm - 1].astype(np.float64)) - np.log(np.float64(K))
        )
    return (-(np.log(fin) + corr))[:, None].astype(np.float32)


TRACE = False
LAST_RESULT = None
LAST_EXEC_S = None
_NC_CACHE = None


def kernel(y_true, y_pred):
    global LAST_RESULT, LAST_EXEC_S, _NC_CACHE
    import time as _time

    P_full, M_full = host_build_inputs(y_true, y_pred)
    if _NC_CACHE is None:
        _NC_CACHE = build_nc()
    nc = _NC_CACHE
    in_maps = [
        {
            "P": np.ascontiguousarray(P_full[c * BPC : (c + 1) * BPC]),
            "M": np.ascontiguousarray(M_full[c * BPC : (c + 1) * BPC]),
        }
        for c in range(NCORES)
    ]
    t0 = _time.time()
    res = run_bass_kernel_spmd(
        nc, in_maps, core_ids=list(range(NCORES)), trace=TRACE
    )
    LAST_EXEC_S = _time.time() - t0
    LAST_RESULT = res
    out = np.empty((B, 1), dtype=np.float32)
    for c in range(NCORES):
        r = res.results[c]
        out[c * BPC : (c + 1) * BPC] = host_finalize(r["XF"], r["CS"])
    return out


# revision 6
# speedup vs baseline: 4241.7656x; 1.2757x over previous
"""CTC loss (keras ctc_batch_cost semantics) on 8 Trainium2 NeuronCores.

Strategy
--------
Pure data parallel over the batch: 8 cores x 64 examples each; no collectives.

The CTC forward recursion runs in the probability domain (not log space):
    alpha_t[s] = (alpha_{t-1}[s] + alpha_{t-1}[s-1] + allow[s]*alpha_{t-1}[s-2])
                 * p_t[ext[s]]
with a rescale every RENORM steps that renormalizes the per-example total to
K = 2**100, keeping the state-profile peak near the top of the fp32 exponent
range so ~150 nats of spread below the peak stay representable (a plain
renorm-to-1 loses ~1% of the probability mass to underflow; log-space per-step
logaddexp is far too slow on this hardware).  The per-step sums C_t come for
free from the final multiply's fused accumulator; the host reassembles
    loss = -(log(alpha_T[S-1] + alpha_T[S-2]) - logK + sum_k log(C_k/K))
in float64.  Storage is bf16 (DVE computes in fp32 internally; bf16 keeps the
fp32 exponent range), measured end-to-end max rel err ~1.1e-4.

Device layout: batch in partitions (64 rows), states along the free dim with
2 zero-pad columns so the s-1/s-2 shifts are plain AP offsets.  Each step is
4 fused scalar_tensor_tensor ops on the vector engine.  The gathered
probability tensor P[b, t, s] = y_pred[b, t, ext[b, s]] + eps is built on the
host (per-partition gathers are not expressible on-device: indirect_copy
shares its index stream across each 16-partition group) and streamed to the
device in double-buffered time chunks; P rows are padded to an even length so
per-step slices stay 4-byte aligned for the DVE 2x bf16 mode.
"""

import ml_dtypes
import numpy as np

import concourse.bacc as bacc
import concourse.bass as bass
import concourse.tile as tile
from concourse import mybir
from concourse.bass_utils import run_bass_kernel_spmd

B, T, C, L = 512, 512, 128, 64
S = 2 * L + 1
SP = S + 1             # P row padded to even length so per-step offsets stay 4B-aligned
BLANK = C - 1
EPS = 1e-7
NCORES = 8
BPC = B // NCORES
RENORM = 4
K = float(2.0 ** 100)
LOG_K = 100.0 * float(np.log(2.0))

F32 = mybir.dt.float32
BF16 = mybir.dt.bfloat16
MULT = mybir.AluOpType.mult
ADD = mybir.AluOpType.add


def build_nc(T_=T, TB=64, bpc=BPC, renorm=RENORM):
    nc = bacc.Bacc(
        "TRN2", target_bir_lowering=False, debug=False, num_devices=NCORES
    )
    P = nc.dram_tensor("P", [bpc, T_, SP], BF16, kind="ExternalInput")
    M = nc.dram_tensor("M", [bpc, S - 2], BF16, kind="ExternalInput")
    XF = nc.dram_tensor("XF", [bpc, 2], F32, kind="ExternalOutput")
    CS = nc.dram_tensor("CS", [bpc, T_], F32, kind="ExternalOutput")

    Pap, Map, XFap, CSap = P.ap(), M.ap(), XF.ap(), CS.ap()
    nchunks = T_ // TB

    with tile.TileContext(nc) as tc:
        with (
            tc.tile_pool(name="persist", bufs=1) as pers,
            tc.tile_pool(name="pchunks", bufs=2) as pp,
        ):
            X = pers.tile([bpc, S + 2], BF16)
            W = pers.tile([bpc, S], BF16)
            G = pers.tile([bpc, S - 2], BF16)
            m = pers.tile([bpc, S - 2], BF16)
            Cs = pers.tile([bpc, T_], F32)
            rc = pers.tile([bpc, 1], F32)
            xf32 = pers.tile([bpc, 2], F32)

            nc.vector.memset(X, 0.0)
            nc.vector.memset(Cs, 0.0)
            nc.sync.dma_start(out=m, in_=Map)

            for k in range(nchunks):
                pch = pp.tile([bpc, TB, SP], BF16, tag="pch")
                nc.sync.dma_start(out=pch, in_=Pap[:, k * TB : (k + 1) * TB, :])

                for i in range(TB):
                    tau = k * TB + i
                    pt = pch[:, i, 0:S]
                    if tau == 0:
                        nc.vector.tensor_scalar_mul(X[:, 2:4], pt[:, 0:2], K)
                        nc.vector.tensor_reduce(
                            Cs[:, 0:1], X[:, 2:4], axis=mybir.AxisListType.X, op=ADD
                        )
                        continue
                    renorm_step = tau % renorm == 0
                    feeds_renorm = (tau + 1) % renorm == 0 and tau + 1 < T_
                    if renorm_step:
                        nc.vector.reciprocal(rc, Cs[:, tau - 1 : tau])
                        nc.vector.tensor_scalar_mul(rc, rc, K)
                    # plain tensor_tensor where no scalar/accum is needed: TT has
                    # a bf16 2x_1p uop on HW; scalar_tensor_tensor may not.
                    nc.vector.tensor_add(W, X[:, 1 : S + 1], X[:, 2 : S + 2])
                    nc.vector.tensor_mul(G, X[:, 2:S], m)
                    nc.vector.tensor_add(W[:, 2:S], G, W[:, 2:S])
                    if renorm_step or feeds_renorm:
                        nc.vector.scalar_tensor_tensor(
                            X[:, 2 : S + 2],
                            W,
                            rc[:, :] if renorm_step else 1.0,
                            pt,
                            op0=MULT,
                            op1=MULT,
                            accum_out=Cs[:, tau : tau + 1] if feeds_renorm else None,
                        )
                    else:
                        nc.vector.tensor_mul(X[:, 2 : S + 2], W, pt)

            # upconvert the two final states to f32 for output
            nc.vector.tensor_copy(xf32, X[:, S : S + 2])
            nc.sync.dma_start(out=XFap, in_=xf32)
            nc.sync.dma_start(out=CSap, in_=Cs)

    nc.compile()
    return nc


def host_build_inputs(y_true, y_pred, T_=T):
    y_true = np.asarray(y_true).astype(np.int64)
    y_pred = np.asarray(y_pred).astype(np.float32)
    Bn = y_true.shape[0]
    p = y_pred + np.float32(EPS)
    ext = np.full((Bn, S), BLANK, dtype=np.int64)
    ext[:, 1::2] = y_true
    allow = np.zeros((Bn, S), dtype=bool)
    allow[:, 2:] = (ext[:, 2:] != BLANK) & (ext[:, 2:] != ext[:, :-2])
    P_full = np.zeros((Bn, T_, SP), dtype=ml_dtypes.bfloat16)
    P_full[:, :, :S] = np.take_along_axis(
        p[:, :T_, :], np.broadcast_to(ext[:, None, :], (Bn, T_, S)), axis=2
    ).astype(ml_dtypes.bfloat16)
    M_full = np.ascontiguousarray(allow[:, 2:].astype(ml_dtypes.bfloat16))
    return P_full, M_full


def host_finalize(XF, CS, T_=T, renorm=RENORM):
    fin = XF[:, 0].astype(np.float64) + XF[:, 1].astype(np.float64)
    corr = -np.log(np.float64(K))
    for k in range(1, T_ // renorm):
        corr = corr + (
            np.log(CS[:, k * renorm - 1].astype(np.float64)) - np.log(np.float64(K))
        )
    return (-(np.log(fin) + corr))[:, None].astype(np.float32)


TRACE = False
LAST_RESULT = None
LAST_EXEC_S = None
_NC_CACHE = None


def kernel(y_true, y_pred):
    global LAST_RESULT, LAST_EXEC_S, _NC_CACHE
    import time as _time

    P_full, M_full = host_build_inputs(y_true, y_pred)
    if _NC_CACHE is None:
        _NC_CACHE = build_nc()
    nc = _NC_CACHE
    in_maps = [
        {
            "P": np.ascontiguousarray(P_full[c * BPC : (c + 1) * BPC]),
            "M": np.ascontiguousarray(M_full[c * BPC : (c + 1) * BPC]),
        }
        for c in range(NCORES)
    ]
    t0 = _time.time()
    res = run_bass_kernel_spmd(
        nc, in_maps, core_ids=list(range(NCORES)), trace=TRACE
    )
    LAST_EXEC_S = _time.time() - t0
    LAST_RESULT = res
    out = np.empty((B, 1), dtype=np.float32)
    for c in range(NCORES):
        r = res.results[c]
        out[c * BPC : (c + 1) * BPC] = host_finalize(r["XF"], r["CS"])
    return out
